# revision 1
# baseline (speedup 1.0000x reference)
"""DisConv GNN message-passing kernel for 8 Trainium2 NeuronCores.

Problem: Z = l2norm(features @ W_k + b_k); 4 iterations of
  att[k] = softmax_k(mask * (Z_k Z_k^T)); Z = l2norm(Z + att @ Z)
Output: [N, K*D] channel-concat.

Strategy (row sharding, N=2048 over 8 cores, 256 rows each):
- Each core holds the full replicated Z in bf16 in two layouts:
  ZT (channel-major [32c x 2048n] stacks of 4 channels) for score matmuls,
  Znm (n-major [128, 16blk*8k*32c]) for aggregation matmuls.
  The core's own 256-column f32 state never leaves the core.
- Per m-block [128m x 256n]: 8 score matmuls (D=32 contraction), one exp
  per channel-stack (ScalarE), bf16 pairwise-tree denominator, approx
  reciprocal, mask*recip, one broadcast multiply -> att; 8 col-packed
  aggregation matmuls accumulate over the 16 m-blocks in PSUM.
- Softmax restructuring: softmax input masking is k-independent, so
  att = mask * exp(S) / sum_k exp(S) exactly matches the reference.
- l2norm via rnorm = exp(-0.5*ln(s^2+eps)) (Ln+Exp share an ACT table set).
- Between iterations: two AllGathers of 128KB/rank bf16 (channel-major ZT
  first so next-iteration scores start early; n-major for aggregation
  second), then 12 large strided reload DMAs.
- HW-verified constraint: matmuls/transposes at different PE row groups run
  concurrently and crash if they write the same PSUM bank -- every
  concurrent row group gets its own bank (4x one-bank score tiles; agg uses
  col-groups, which write distinct partitions and may share banks).
- Engine balance per block (~2.1-2.3 us each): ScalarE 4 exps; DVE tree
  lvl1 + approx-reciprocal + 2-way-split att multiply; GPSIMD tree lvl2 +
  den + mask*recip; PE 8 score + 8 agg matmuls.
- Measured ~370-440 ns*1e3 per execution (rep-slope method; see test.py),
  vs reference single-device JAX at several ms.
"""

import sys

sys.path.insert(0, "/opt/trn_rl_repo")

import numpy as np
import ml_dtypes

N = 2048
IN_DIM = 128
K = 8
D = 32
ITERS = 4
NCORES = 8
NLOC = N // NCORES  # 256
NBLK = N // 128  # 16
EPS2 = 1e-24

BF = ml_dtypes.bfloat16

_compiled = None


DEFAULT_CFG = dict(
    score_tiles=4,   # 4 one-bank score tiles vs 2 two-bank tiles
    lvl2_eng="gpsimd",
    den_eng="gpsimd",
    rm_eng="gpsimd",
    eall_bufs=3,
    att_bufs=3,
    pipe_bufs=3,
    score_imajor=True,
    att_split=2,
    tree_split=False,
    agg_slotmajor=False,
)


def _build(reps=1, sim_mode=False, cfg=None):
    """sim_mode: single-core, collective replaced by a DRAM->DRAM DMA with
    the same dependency shape, for TimelineSim cost-model iteration."""
    import concourse.bacc as bacc
    import concourse.mybir as mybir
    from concourse import tile

    # The ACT table-load pass picks the first set containing each function,
    # which puts Exp (set 0) and Ln (set 5) in different table sets and
    # reloads tables twice per iteration boundary (~2.7us each). Restrict
    # Exp/Ln to natural_log_exp_and_others (which holds both) so one load
    # serves the whole kernel. Indices/order are preserved.
    if not getattr(bacc, "_dis_act_tables_patched", False):
        _orig_tabs = bacc.get_activation_tables

        def _patched_tabs(arch, _orig=_orig_tabs, _AF=mybir.ActivationFunctionType):
            out = {}
            for name, fns in _orig(arch).items():
                fns = set(fns)
                if name != "natural_log_exp_and_others":
                    fns.discard(_AF.Exp)
                    fns.discard(_AF.Ln)
                out[name] = fns
            return out

        bacc.get_activation_tables = _patched_tabs
        bacc._dis_act_tables_patched = True

    cfg = {**DEFAULT_CFG, **(cfg or {})}

    f32 = mybir.dt.float32
    bf16 = mybir.dt.bfloat16
    AF = mybir.ActivationFunctionType
    ALU = mybir.AluOpType

    nc = bacc.Bacc(
        "TRN2",
        target_bir_lowering=False,
        debug=False,
        num_devices=1 if sim_mode else NCORES,
    )
    nc._dis_sim_mode = sim_mode
    nc._dis_cfg = cfg

    # ---- I/O -------------------------------------------------------------
    featT_loc = nc.dram_tensor("featT_loc", [IN_DIM, NLOC], f32, kind="ExternalInput")
    maskT_in = nc.dram_tensor("maskT", [NBLK, 128, NLOC], bf16, kind="ExternalInput")
    wstack_in = nc.dram_tensor("wstack", [IN_DIM, K * D], f32, kind="ExternalInput")
    bstack_in = nc.dram_tensor("bstack", [128, 2], f32, kind="ExternalInput")
    onesblk_in = nc.dram_tensor("onesblk", [128, 128], f32, kind="ExternalInput")
    id128_in = nc.dram_tensor("id128", [128, 128], f32, kind="ExternalInput")
    out_dram = nc.dram_tensor("out", [2, 128, NLOC], f32, kind="ExternalOutput")

    rg = [list(range(NCORES))]

    with tile.TileContext(nc) as tc:
        with (
            tc.tile_pool(name="const", bufs=1) as constp,
            tc.tile_pool(name="state", bufs=2) as statep,
            tc.tile_pool(name="work", bufs=2) as workp,
            tc.tile_pool(name="psum", bufs=1, space="PSUM") as psp,
            tc.tile_pool(name="psagg", bufs=1, space="PSUM") as psaggp,
            tc.tile_pool(name="dram", bufs=2, space="DRAM") as dramp,
        ):
            # ---- persistent SBUF tensors --------------------------------
            featT = constp.tile([IN_DIM, NLOC], f32)
            nc.sync.dma_start(featT[:], featT_loc[:])
            wstack = constp.tile([IN_DIM, K * D], f32)
            nc.sync.dma_start(wstack[:], wstack_in[:])
            bstack = constp.tile([128, 2], f32)
            nc.sync.dma_start(bstack[:], bstack_in[:])
            onesblk = constp.tile([128, 128], f32)
            nc.sync.dma_start(onesblk[:], onesblk_in[:])
            id128 = constp.tile([128, 128], f32)
            nc.sync.dma_start(id128[:], id128_in[:])
            epsb = constp.tile([128, 1], f32)
            nc.any.memset(epsb[:], EPS2)
            maskT = constp.tile([128, NBLK * NLOC], bf16)
            for q in range(8):
                nc.sync.dma_start(
                    maskT[:, q * 2 * NLOC : (q + 1) * 2 * NLOC].rearrange(
                        "p (b n) -> p b n", b=2
                    ),
                    maskT_in[2 * q : 2 * q + 2].rearrange("b p n -> p b n"),
                )

            # replicated Z (bf16, rebuilt each round via AllGather)
            ZTs = [constp.tile([128, N], bf16, name=f"ZT{s}") for s in range(2)]
            Znm = constp.tile([128, NBLK * K * D], bf16)

            def normalize_and_distribute(zsum, rnd, last):
                """zsum: 2 stacks [128(4ch x 32c), NLOC] f32 (SBUF or PSUM src
                handled by caller adds). Produces:
                - new local f32 state (returned)
                - bf16 local tiles in both layouts, AllGather, reload replicas
                or, if last, writes the output DRAM tensor."""
                # norms for both stacks in one [128, 512] stream: one
                # blockdiag matmul, one Ln, one Exp (shorter boundary chain).
                sq = workp.tile([128, 2 * NLOC], f32, name=f"sq{rnd}", tag="sq")
                for s in range(2):
                    nc.vector.tensor_tensor(
                        sq[:, s * NLOC : (s + 1) * NLOC], zsum[s][:], zsum[s][:], ALU.mult
                    )
                n2tag = "sps0" if cfg["score_tiles"] == 4 else "sps0_1"
                n2 = psp.tile([128, 2 * NLOC], f32, name=f"n2{rnd}", tag=n2tag)
                nc.tensor.matmul(n2[:], onesblk[:], sq[:], start=True, stop=True)
                lg = workp.tile([128, 2 * NLOC], f32, name=f"lg{rnd}", tag="lg")
                nc.scalar.activation(lg[:], n2[:], AF.Ln, bias=epsb[:])
                rn = workp.tile([128, 2 * NLOC], f32, name=f"rn{rnd}", tag="rn", bufs=3)
                nc.scalar.activation(rn[:], lg[:], AF.Exp, scale=-0.5)
                zloc = []
                rnorms = []
                for s in range(2):
                    rns = rn[:, s * NLOC : (s + 1) * NLOC]
                    zn = statep.tile([128, NLOC], f32, name=f"zloc{rnd}{s}", tag=f"zloc{s}")
                    nc.vector.tensor_tensor(zn[:], zsum[s][:], rns, ALU.mult)
                    zloc.append(zn)
                    rnorms.append(rns)

                # transpose local columns to n-major: 2 chunks of 128 rows.
                # Row-group-concurrent PE ops must write different PSUM banks,
                # so each row group i gets its own one-bank tile; stack s picks
                # the column half. Channel 4s+i lands at slot 2i+s ("slot
                # order"), which the E/att/Znm replica layouts share.
                # Row-group-concurrent transposes need 4 distinct PSUM banks:
                # reuse the score tags (4x one-bank, or 2x two-bank) so the
                # boundary shares the score-tile PSUM budget.
                four = cfg["score_tiles"] == 4
                pst = []
                for c in range(2):
                    if four:
                        pt = [
                            psp.tile([128, 64], f32, name=f"pt{rnd}{c}{i}", tag=f"sps{i}")
                            for i in range(4)
                        ]

                        def pslice(i, s, w=32, pt=pt):
                            return pt[i][:, s * 32 : s * 32 + w]
                    else:
                        pt = [
                            psp.tile(
                                [128, 4 * NLOC], f32, name=f"pt{rnd}{c}{h}",
                                tag=f"sps{2 * h}_{2 * h + 1}",
                            )
                            for h in range(2)
                        ]

                        def pslice(i, s, w=32, pt=pt):
                            base = (i % 2) * 512 + s * 32
                            return pt[i // 2][:, base : base + w]

                    for s in range(2):
                        for i in range(4):
                            nc.tensor.transpose(
                                pslice(i, s),
                                zloc[s][32 * i : 32 * (i + 1), c * 128 : (c + 1) * 128],
                                id128[32 * i : 32 * (i + 1), 32 * i : 32 * (i + 1)],
                                tile_position=(32 * i, 0),
                            )
                    pst.append((pt, pslice))

                if last:
                    for c in range(2):
                        ot = workp.tile([128, 256], f32, name=f"ot{c}", tag="ot")
                        _, psl = pst[c]
                        for s in range(2):
                            for i in range(4):
                                k = 4 * s + i
                                nc.scalar.copy(ot[:, k * 32 : (k + 1) * 32], psl(i, s))
                        nc.sync.dma_start(out_dram[c], ot[:])
                    return zloc

                sim = getattr(nc, "_dis_sim_mode", False)
                shared = "Local" if sim else "Shared"

                def allgather(agin, agout):
                    if sim:  # stand-in with the same dependency shape
                        for r in range(NCORES):
                            nc.sync.dma_start(agout[r], agin[:])
                    else:
                        nc.gpsimd.collective_compute(
                            "AllGather",
                            mybir.AluOpType.bypass,
                            replica_groups=rg,
                            ins=[agin[:].opt()],
                            outs=[agout[:].opt()],
                        )

                # AG1: channel-major ZT replicas. ztl is computed directly as
                # (zsum*rnorm) -> bf16 so the AG1 path skips a cast hop.
                ztl = []
                for s in range(2):
                    t = statep.tile([128, NLOC], bf16, name=f"ztl{rnd}{s}", tag=f"ztl{s}")
                    nc.vector.tensor_tensor(t[:], zsum[s][:], rnorms[s], ALU.mult)
                    ztl.append(t)
                agin1 = dramp.tile([2, 128, 256], bf16, name=f"agin1_{rnd}", tag="agin1")
                agout1 = dramp.tile(
                    [NCORES, 2, 128, 256], bf16,
                    name=f"agout1_{rnd}", tag="agout1", addr_space=shared,
                )
                for s in range(2):
                    nc.sync.dma_start(agin1[s], ztl[s][:])
                allgather(agin1, agout1)
                # per-rank-pair reloads, rank-major so early blocks start first
                for g in range(4):
                    for s in range(2):
                        nc.sync.dma_start(
                            ZTs[s][:, g * 512 : (g + 1) * 512].rearrange(
                                "p (r n) -> p r n", r=2
                            ),
                            agout1[2 * g : 2 * g + 2, s].rearrange("r p n -> p r n"),
                        )

                # AG2: n-major replicas (needed only by the aggregation stage)
                znml = []
                for c in range(2):
                    t = workp.tile([128, 256], bf16, name=f"znml{rnd}{c}", tag=f"znml{c}")
                    _, psl = pst[c]
                    for i in range(4):
                        nc.vector.tensor_copy(t[:, i * 64 : (i + 1) * 64], psl(i, 0, 64))
                    znml.append(t)
                agin2 = dramp.tile([2, 128, 256], bf16, name=f"agin2_{rnd}", tag="agin2")
                agout2 = dramp.tile(
                    [NCORES, 2, 128, 256], bf16,
                    name=f"agout2_{rnd}", tag="agout2", addr_space=shared,
                )
                for c in range(2):
                    nc.sync.dma_start(agin2[c], znml[c][:])
                allgather(agin2, agout2)
                for g in range(4):
                    nc.sync.dma_start(
                        Znm[:, g * 1024 : (g + 1) * 1024].rearrange(
                            "p (q n) -> p q n", q=4
                        ),
                        agout2[2 * g : 2 * g + 2].rearrange("j c p n -> p (j c) n"),
                    )
                return zloc, ztl

            # ---- init: Z0 = l2norm(features @ W + b) for local columns ---
            for rep in range(reps):
                _body_once(
                    nc, tc, tile, mybir, rep,
                    featT, wstack, bstack, onesblk, id128, epsb, maskT, ZTs, Znm,
                    statep, workp, psp, psaggp, dramp, out_dram, rg,
                    normalize_and_distribute,
                )

    nc.compile()
    return nc


def _body_once(
    nc, tc, tile, mybir, rep,
    featT, wstack, bstack, onesblk, id128, epsb, maskT, ZTs, Znm,
    statep, workp, psp, psaggp, dramp, out_dram, rg,
    normalize_and_distribute,
):
    f32 = mybir.dt.float32
    bf16 = mybir.dt.bfloat16
    AF = mybir.ActivationFunctionType
    ALU = mybir.AluOpType
    if True:
        if True:
            zsum0 = []
            for s in range(2):
                cfg = nc._dis_cfg
                iptag = f"sps{s}" if cfg["score_tiles"] == 4 else f"sps{2 * s}_{2 * s + 1}"
                ip = psp.tile([128, NLOC], f32, name=f"initp{rep}{s}", tag=iptag)
                for i in range(4):
                    nc.tensor.matmul(
                        ip[32 * i : 32 * (i + 1), :],
                        wstack[:, (4 * s + i) * D : (4 * s + i + 1) * D],
                        featT[:],
                        start=True,
                        stop=True,
                        tile_position=(0, 32 * i),
                    )
                zs = workp.tile([128, NLOC], f32, name=f"zsum0{s}", tag="zsum")
                nc.vector.tensor_scalar(
                    zs[:], ip[:], bstack[:, s : s + 1], None, ALU.add
                )
                zsum0.append(zs)
            zloc, ztl = normalize_and_distribute(zsum0, 0, last=False)

            # ---- iterations ---------------------------------------------
            for it in range(ITERS):
                aggps = [
                    psaggp.tile([128, NLOC], f32, name=f"agg{it}{s}", tag=f"agg{s}")
                    for s in range(2)
                ]
                cfg = nc._dis_cfg
                for blk in range(NBLK):
                    # Score tiles. Concurrent row groups must hit distinct
                    # PSUM banks. Channel 4s+i at E slot 2i+s ("slot order").
                    # score_tiles=4: one bank per row group (finer pipeline
                    # rotation); score_tiles=2: two 2-bank tiles (fewer exps).
                    if cfg["score_tiles"] == 4:
                        sps = [
                            psp.tile(
                                [128, 2 * NLOC], f32, name=f"sps{it}{blk}{i}", tag=f"sps{i}"
                            )
                            for i in range(4)
                        ]
                        def sslice(s, i):
                            return sps[i][:, s * NLOC : (s + 1) * NLOC]
                        exps = [(sps[i][:], i * 512) for i in range(4)]
                    else:
                        sps = [
                            psp.tile(
                                [128, 4 * NLOC], f32, name=f"sps{it}{blk}{h}",
                                tag=f"sps{2 * h}_{2 * h + 1}",
                            )
                            for h in range(2)
                        ]
                        def sslice(s, i):
                            base = (i % 2) * 2 * NLOC + s * NLOC
                            return sps[i // 2][:, base : base + NLOC]
                        exps = [(sps[h][:], h * 1024) for h in range(2)]
                    order = (
                        [(s, i) for i in range(4) for s in range(2)]
                        if cfg["score_imajor"]
                        else [(s, i) for s in range(2) for i in range(4)]
                    )
                    for s, i in order:
                        nc.tensor.matmul(
                            sslice(s, i),
                            ZTs[s][32 * i : 32 * (i + 1), blk * 128 : (blk + 1) * 128],
                            ztl[s][32 * i : 32 * (i + 1), :],
                            start=True,
                            stop=True,
                            tile_position=(32 * i, 0),
                        )
                    eall = workp.tile(
                        [128, K * NLOC], bf16, name=f"eall{it}{blk}", tag="eall",
                        bufs=cfg["eall_bufs"],
                    )
                    for src_ap, col in exps:
                        nc.scalar.activation(
                            eall[:, col : col + src_ap.shape[-1]], src_ap, AF.Exp
                        )
                    # denominator tree: 3 ops
                    t1 = workp.tile([128, 1024], bf16, name=f"t1_{it}{blk}", tag="t1", bufs=cfg["pipe_bufs"])
                    if cfg["tree_split"]:
                        for h in range(2):
                            evh = eall[:, h * 1024 : (h + 1) * 1024].rearrange(
                                "p (a n) -> p a n", a=2
                            )
                            t1h = t1[:, h * 512 : (h + 1) * 512].rearrange(
                                "p (a n) -> p a n", a=2
                            )
                            nc.vector.tensor_tensor(
                                t1h, evh[:, :, 0:NLOC], evh[:, :, NLOC : 2 * NLOC], ALU.add
                            )
                    else:
                        ev = eall[:].rearrange("p (a n) -> p a n", a=4)
                        t1v = t1[:].rearrange("p (a n) -> p a n", a=4)
                        nc.vector.tensor_tensor(
                            t1v, ev[:, :, 0:NLOC], ev[:, :, NLOC : 2 * NLOC], ALU.add
                        )
                    t2 = workp.tile([128, 512], bf16, name=f"t2_{it}{blk}", tag="t2", bufs=cfg["pipe_bufs"])
                    t1w = t1[:].rearrange("p (a n) -> p a n", a=2)
                    t2v = t2[:].rearrange("p (a n) -> p a n", a=2)
                    eng2 = nc.gpsimd if cfg["lvl2_eng"] == "gpsimd" else nc.vector
                    eng2.tensor_tensor(
                        t2v, t1w[:, :, 0:NLOC], t1w[:, :, NLOC : 2 * NLOC], ALU.add
                    )
                    den = workp.tile([128, NLOC], f32, name=f"den{it}{blk}", tag="den", bufs=cfg["pipe_bufs"])
                    engd = nc.gpsimd if cfg["den_eng"] == "gpsimd" else nc.vector
                    engd.tensor_tensor(
                        den[:], t2[:, 0:NLOC], t2[:, NLOC : 2 * NLOC], ALU.add
                    )
                    rcp = workp.tile([128, NLOC], f32, name=f"rcp{it}{blk}", tag="rcp", bufs=cfg["pipe_bufs"])
                    nc.vector.reciprocal_approx_fast(rcp[:], den[:])
                    rmask = workp.tile([128, NLOC], bf16, name=f"rm{it}{blk}", tag="rm", bufs=cfg["pipe_bufs"])
                    engr = nc.gpsimd if cfg["rm_eng"] == "gpsimd" else nc.vector
                    engr.tensor_tensor(
                        rmask[:], rcp[:], maskT[:, blk * NLOC : (blk + 1) * NLOC], ALU.mult
                    )
                    att = workp.tile(
                        [128, K * NLOC], bf16, name=f"att{it}{blk}", tag="att", bufs=cfg["att_bufs"]
                    )
                    nsp = cfg["att_split"]
                    kk = K // nsp
                    for h in range(nsp):
                        lo = h * kk * NLOC
                        hi = (h + 1) * kk * NLOC
                        nc.vector.tensor_tensor(
                            att[:, lo:hi].rearrange("p (a n) -> p a n", a=kk),
                            eall[:, lo:hi].rearrange("p (a n) -> p a n", a=kk),
                            rmask[:, None, :].to_broadcast((128, kk, NLOC)),
                            ALU.mult,
                        )
                    if cfg["agg_slotmajor"]:
                        agord = [(slot % 2, slot // 2) for slot in range(8)]
                    else:
                        agord = [(s, i) for s in range(2) for i in range(4)]
                    for s, i in agord:
                        slot = 2 * i + s  # channel 4s+i in replica layouts
                        nc.tensor.matmul(
                            aggps[s][32 * i : 32 * (i + 1), :],
                            Znm[:, (blk * K + slot) * D : (blk * K + slot + 1) * D],
                            att[:, slot * NLOC : (slot + 1) * NLOC],
                            start=(blk == 0),
                            stop=(blk == NBLK - 1),
                            tile_position=(0, 32 * i),
                            skip_group_check=True,
                        )
                # residual + renorm + redistribute
                zsum = []
                for s in range(2):
                    zs = workp.tile([128, NLOC], f32, name=f"zsum{it}{s}", tag="zsum")
                    nc.vector.tensor_tensor(zs[:], zloc[s][:], aggps[s][:], ALU.add)
                    zsum.append(zs)
                if it == ITERS - 1:
                    normalize_and_distribute(zsum, it + 1, last=True)
                else:
                    zloc, ztl = normalize_and_distribute(zsum, it + 1, last=False)


def _prep_inputs(adj, features, W, b):
    adj = np.asarray(adj)
    features = np.asarray(features, np.float32)
    W = np.asarray(W, np.float32)
    b = np.asarray(b, np.float32)

    wstack = np.ascontiguousarray(W.transpose(1, 0, 2).reshape(IN_DIM, K * D))
    bstack = np.zeros((128, 2), np.float32)
    for s in range(2):
        for i in range(4):
            bstack[32 * i : 32 * (i + 1), s] = b[4 * s + i]
    onesblk = np.zeros((128, 128), np.float32)
    for j in range(4):
        onesblk[32 * j : 32 * (j + 1), 32 * j : 32 * (j + 1)] = 1.0
    id128 = np.eye(128, dtype=np.float32)

    in_maps = []
    for c in range(NCORES):
        rows = slice(c * NLOC, (c + 1) * NLOC)
        featT_loc = np.ascontiguousarray(features[rows].T)
        maskT = (adj[rows].T > 0).astype(np.float32).astype(BF)
        maskT = np.ascontiguousarray(maskT.reshape(NBLK, 128, NLOC))
        in_maps.append(
            {
                "featT_loc": featT_loc,
                "maskT": maskT,
                "wstack": wstack,
                "bstack": bstack,
                "onesblk": onesblk,
                "id128": id128,
            }
        )
    return in_maps


def run(adj, features, W, b, trace=False, **trace_kwargs):
    global _compiled
    if _compiled is None:
        _compiled = _build()
    from concourse import bass_utils

    in_maps = _prep_inputs(adj, features, W, b)
    res = bass_utils.run_bass_kernel_spmd(
        _compiled, in_maps, core_ids=list(range(NCORES)), trace=trace, **trace_kwargs
    )
    outs = [res.results[c]["out"].reshape(NLOC, NLOC) for c in range(NCORES)]
    full = np.concatenate(outs, axis=0)
    return full, res


def kernel(adj, features, W, b):
    full, _ = run(adj, features, W, b, trace=False)
    return full

